# revision 21
# baseline (speedup 1.0000x reference)
"""FPN ROIAlign pooler (nn_Pooler) on 8 trn2 cores — matmul formulation.

Strategy: data-parallel over RoIs. Host builds a channels-last fp16 pixel
table [px, 256ch] (row-major feature pyramid + an x-major copy of lvl0 for
tall windows). Per RoI the bounding rect of its bilinear taps becomes a
list of 6-px strips (one DMA descriptor each; dma_gather costs ~8.4ns per
descriptor on gpsimd, so 6-px strips balance descriptor count against
fetch padding). Overlapping same-level windows merge into union-rect jobs
(two RoIs sharing strips). Jobs split into pieces of <=128 strips; pieces
pack two-per-slot (one per 49-bin side). Every slot is one gathered chunk
[128 strips, 6*256] = 6 matmul sub-chunks [128 px, 256] with px on
partitions. ROIAlign weights are separable (W[px,bin] = Ay[y,by]*Ax[x,bx]);
host streams per-strip-sub factors [AyA|AyB|AxA|AxB] (4*7 fp16) and the
DVE forms the slot's 98-wide W on device. The PE array accumulates
pooled[98, 256] = W^T @ F in PSUM over the slot's 6 sub-chunks; the scalar
engine copies PSUM -> SBUF as fp16 and rows DMA out. Host adds piece
partial sums and reassembles [1024,256,7,7] in fp32.

int16 gather indices limit a gather group to <32768 px, so lvl0 (60800 px
per batch) is covered by 3 overlapping y-bands (windows with winH<=54) and
3 overlapping x-bands of the transposed copy (winH>54 implies winW<=17,
since window feature area <= 784 + margins). HW limits one dma_gather to
<=1024 descriptors = exactly one 8-slot wave.
"""
import numpy as np
from contextlib import ExitStack

from concourse import bacc, bass, mybir, tile, bass_utils

C = 256
N_CORES = 8
OUT = 7
S = 6               # strip length (px per gather descriptor)
LVL_HW = [(200, 304), (100, 152), (50, 76), (25, 38)]
SCALES = (0.25, 0.125, 0.0625, 0.03125)
SEG_SZ = [h * w for h, w in LVL_HW]           # px per (lvl, batch) segment
# segment order: (0,0),(0,1),(1,0),(1,1),(2,0),(2,1),(3,0),(3,1)
SEG_BASE = np.zeros((4, 2), np.int64)
_off = 0
for _l in range(4):
    for _b in range(2):
        SEG_BASE[_l, _b] = _off
        _off += SEG_SZ[_l]
TOTAL_PX = int(_off)                           # 161500
XSEG_BASE = [TOTAL_PX, TOTAL_PX + SEG_SZ[0]]   # x-major lvl0 copies
END_PAD = 32
TABLE_PX = TOTAL_PX + 2 * SEG_SZ[0] + END_PAD

ROW_BANDS = [(0, 107), (53, 107), (106, 94)]   # (row0, nrows), lvl0 W=304
XCOL_BANDS = [(0, 107), (90, 107), (180, 124)]  # (col0, ncols), lvl0 H=200
HMAX_ROW = 54        # row-major path requires winH <= 54
WMAX_X = 17          # x-major path requires winW <= 17

# gather groups: (table_base_px, n_px)
GROUPS = []
for _b in range(2):
    for _r0, _nr in ROW_BANDS:
        GROUPS.append((int(SEG_BASE[0, _b]) + _r0 * 304, _nr * 304))
for _b in range(2):
    for _c0, _ncol in XCOL_BANDS:
        GROUPS.append((XSEG_BASE[_b] + _c0 * 200, _ncol * 200))
GROUPS.append((int(SEG_BASE[1, 0]), 2 * SEG_SZ[1]))             # lvl1 both b
GROUPS.append((int(SEG_BASE[2, 0]), 2 * SEG_SZ[2] + 2 * SEG_SZ[3]))  # lvl2+3
NG = len(GROUPS)                               # 14
for _base, _npx in GROUPS:
    assert _npx <= 32768

MAXW = 124          # max window extent per side
AROWS = MAXW + S + 2

_nc_cache = {}


def _build_nc(schedule):
    """schedule: tuple of (g, n_slots)."""
    tot_slots = sum(n for _, n in schedule)
    tot_idxc = tot_slots * 8
    tot_acols = tot_slots * S * 28

    nc = bacc.Bacc("TRN2", target_bir_lowering=False, debug=False,
                   num_devices=N_CORES, num_swdge_queues=2)
    table_d = nc.dram_tensor("table", [TABLE_PX * C], mybir.dt.float16,
                             kind="ExternalInput")
    idx_d = nc.dram_tensor("idxs", [128, tot_idxc], mybir.dt.int16,
                           kind="ExternalInput")
    a_d = nc.dram_tensor("afac", [128, tot_acols], mybir.dt.float16,
                         kind="ExternalInput")
    # bin-major output: [98 bins, tot_slots * C] -> one 4KB-descriptor
    # DMA per wave instead of 98 tiny rows per slot
    out_d = nc.dram_tensor("out", [98, tot_slots * C], mybir.dt.float16,
                           kind="ExternalOutput")

    with tile.TileContext(nc) as tc, ExitStack() as ctx:
        sbi = ctx.enter_context(tc.tile_pool(name="sbi", bufs=6))
        sba = ctx.enter_context(tc.tile_pool(name="sba", bufs=6))
        sbd = ctx.enter_context(tc.tile_pool(name="sbd", bufs=6))
        sbw = ctx.enter_context(tc.tile_pool(name="sbw", bufs=6))
        sbo = ctx.enter_context(tc.tile_pool(name="sbo", bufs=4))
        psp = ctx.enter_context(tc.tile_pool(name="psp", bufs=8,
                                             space="PSUM"))

        # wave list in slot order, then interleave small (low PE work)
        # waves among full ones so the PE never starves
        waves = []
        ri = 0
        for g, nslots in schedule:
            for pos in range(0, nslots, 4):
                wch = min(4, nslots - pos)
                waves.append((g, ri, wch))
                ri += wch
        bigs = [w for w in waves if w[2] == 4]
        smalls = [w for w in waves if w[2] < 4]
        order = []
        bi = si = 0
        while bi < len(bigs) or si < len(smalls):
            for _ in range(2):
                if bi < len(bigs):
                    order.append(bigs[bi]); bi += 1
            if si < len(smalls):
                order.append(smalls[si]); si += 1

        for wv, (g, ri, wch) in enumerate(order):
            base_px, n_px = GROUPS[g]
            in_ap = bass.AP(tensor=table_d, offset=base_px * C,
                            ap=[[C, n_px], [1, S * C]])
            if True:
                idx_t = sbi.tile([128, 4 * 8], mybir.dt.int16)
                nc.default_dma_engine.dma_start(
                    out=idx_t[:, :wch * 8],
                    in_=idx_d.ap()[:, ri * 8:(ri + wch) * 8])
                dst_t = sbd.tile([128, 4, S * C], mybir.dt.float16)
                nc.gpsimd.dma_gather(
                    dst_t[:, :wch, :], in_ap, idx_t[:, :wch * 8],
                    wch * 128, wch * 128, S * C, elem_step=C,
                    queue_num=wv % 2)
                a_t = sba.tile([128, 4 * S, 28], mybir.dt.float16)
                nc.default_dma_engine.dma_start(
                    out=a_t[:, :wch * S, :],
                    in_=a_d.ap()[:, ri * S * 28:(ri + wch) * S * 28])
                w_t = sbw.tile([128, 4 * S, 2, 49], mybir.dt.float16)
                for j in range(2):
                    nc.vector.tensor_tensor(
                        out=w_t[:, :wch * S, j, :].rearrange(
                            "p c (a b) -> p c a b", a=7, b=7),
                        in0=a_t[:, :wch * S, 7 * j:7 * j + 7]
                        .unsqueeze(3).broadcast_to([128, wch * S, 7, 7]),
                        in1=a_t[:, :wch * S, 14 + 7 * j:21 + 7 * j]
                        .unsqueeze(2).broadcast_to([128, wch * S, 7, 7]),
                        op=mybir.AluOpType.mult)
                ot = sbo.tile([98, 4, C], mybir.dt.float16)
                for t in range(wch):
                    ps_t = psp.tile([98, 512], mybir.dt.float32)
                    for i in range(S):
                        nc.tensor.matmul(
                            ps_t[:, :C],
                            w_t[:, t * S + i, :, :].rearrange(
                                "p a b -> p (a b)"),
                            dst_t[:, t, i * C:(i + 1) * C],
                            start=(i == 0), stop=(i == S - 1))
                    nc.scalar.copy(out=ot[:, t, :], in_=ps_t[:, :C])
                # issue from the scalar queue: its wait on `ot` is already
                # satisfied there, keeping the sync queue free to prefetch
                # the next wave's idx/afac loads
                nc.scalar.dma_start(
                    out=out_d.ap()[:, ri * C:(ri + wch) * C],
                    in_=ot[:, :wch, :].rearrange("p a b -> p (a b)"))
        assert sum(w[2] for w in order) == tot_slots
    nc.compile()
    return nc


def _host_prep(f0, f1, f2, f3, boxes, bidx):
    boxes32 = np.asarray(boxes, np.float32)
    b = np.asarray(bidx).astype(np.int64)
    N = boxes32.shape[0]

    # level routing in strict fp32 (matches jax reference arithmetic)
    x1, y1, x2, y2 = (boxes32[:, k] for k in range(4))
    area = (x2 - x1 + np.float32(1.0)) * (y2 - y1 + np.float32(1.0))
    s = np.sqrt(area)
    lv = np.floor(np.float32(4.0) + np.log2(s / np.float32(224.0)
                                            + np.float32(1e-6)))
    lvl = (np.clip(lv, 2.0, 5.0)).astype(np.int64) - 2

    # channels-last flat fp16 table (+ x-major lvl0 copies for tall windows)
    segs = []
    for f in (f0, f1, f2, f3):
        fa = np.asarray(f, np.float32)
        for bb in range(2):
            segs.append(np.transpose(fa[bb], (1, 2, 0)).reshape(-1, C))
    fa0 = np.asarray(f0, np.float32)
    for bb in range(2):
        segs.append(np.transpose(fa0[bb], (2, 1, 0)).reshape(-1, C))
    segs.append(np.zeros((END_PAD, C), np.float32))
    table_flat = np.concatenate(segs, 0).astype(np.float16).reshape(-1)
    assert table_flat.size == TABLE_PX * C

    # bilinear sample geometry (fp64 tap positions, like proven baseline)
    scs = np.array(SCALES)[lvl]
    Wl = np.array([hw[1] for hw in LVL_HW])[lvl]
    Hl = np.array([hw[0] for hw in LVL_HW])[lvl]
    x1s = boxes32[:, 0].astype(np.float64) * scs
    y1s = boxes32[:, 1].astype(np.float64) * scs
    x2s = boxes32[:, 2].astype(np.float64) * scs
    y2s = boxes32[:, 3].astype(np.float64) * scs
    bin_w = np.maximum(x2s - x1s, 1.0) / OUT
    bin_h = np.maximum(y2s - y1s, 1.0) / OUT
    grid = (np.arange(OUT)[:, None] + np.array([0.25, 0.75])[None, :]).reshape(-1)
    xs = x1s[:, None] + bin_w[:, None] * grid[None, :]     # [N,14]
    ys = y1s[:, None] + bin_h[:, None] * grid[None, :]
    vx = (xs >= -1.0) & (xs <= Wl[:, None])
    vy = (ys >= -1.0) & (ys <= Hl[:, None])
    xc = np.clip(xs, 0.0, (Wl - 1)[:, None])
    yc = np.clip(ys, 0.0, (Hl - 1)[:, None])
    x0c = np.minimum(np.floor(xc).astype(np.int64), (Wl - 2)[:, None])
    y0c = np.minimum(np.floor(yc).astype(np.int64), (Hl - 2)[:, None])
    lx = xc - x0c
    ly = yc - y0c

    # window rects (inclusive px bounds [rx0..rx1] x [ry0..ry1])
    rx0 = x0c.min(axis=1); rx1 = x0c.max(axis=1) + 1
    ry0 = y0c.min(axis=1); ry1 = y0c.max(axis=1) + 1
    winW = (rx1 - rx0 + 1).astype(np.int64)
    winH = (ry1 - ry0 + 1).astype(np.int64)
    assert winW.max() <= MAXW and winH.max() <= MAXW
    npx = winH * winW

    # separable per-axis weight matrices Ay/Ax [N, AROWS, 7]
    nn = np.broadcast_to(np.arange(N)[:, None], (N, 14))
    sbin = np.broadcast_to((np.arange(14) // 2)[None, :], (N, 14))
    Ay = np.zeros((N, AROWS, OUT))
    Ax = np.zeros((N, AROWS, OUT))
    for kyw, rel, base, vv, rr in ((1.0 - ly, y0c, ry0, vy, Ay),
                                   (1.0 - lx, x0c, rx0, vx, Ax)):
        w0 = kyw * vv * 0.5
        w1 = (1.0 - kyw) * vv * 0.5
        np.add.at(rr, (nn, rel - base[:, None], sbin), w0)
        np.add.at(rr, (nn, rel - base[:, None] + 1, sbin), w1)
    Ay16 = Ay.astype(np.float16)
    Ax16 = Ax.astype(np.float16)

    # gather group per roi
    group = np.empty(N, np.int64)
    xmajor = np.zeros(N, np.bool_)
    for i in range(N):
        lv_, b_ = int(lvl[i]), int(b[i])
        if lv_ == 0:
            if winH[i] <= HMAX_ROW:
                band = 0 if ry1[i] <= 106 else (1 if ry1[i] <= 159 else 2)
                assert ry0[i] >= ROW_BANDS[band][0]
                assert ry1[i] <= ROW_BANDS[band][0] + ROW_BANDS[band][1] - 1
                group[i] = b_ * 3 + band
            else:
                assert winW[i] <= WMAX_X
                band = 0 if rx1[i] <= 106 else (1 if rx1[i] <= 196 else 2)
                assert rx0[i] >= XCOL_BANDS[band][0]
                assert rx1[i] <= XCOL_BANDS[band][0] + XCOL_BANDS[band][1] - 1
                group[i] = 6 + b_ * 3 + band
                xmajor[i] = True
        elif lv_ == 1:
            group[i] = 12
        else:
            group[i] = 13

    def nstrips(od, cd):
        return od * (-(-cd // S))

    # ---- union matching (global, per group; same (lvl,b) overlap) ----
    jobs = []   # (g, ra, rb, rect) ; rb=-1 single. rect=(qy0,qy1,qx0,qx1)
    for g in range(NG):
        ids = sorted([i for i in range(N) if group[i] == g],
                     key=lambda i: -npx[i])
        used = set()
        for a_i in ids:
            if a_i in used:
                continue
            used.add(a_i)
            bestp = None
            for b_i in ids:
                if b_i in used or lvl[a_i] != lvl[b_i] or b[a_i] != b[b_i]:
                    continue
                ox = min(rx1[a_i], rx1[b_i]) - max(rx0[a_i], rx0[b_i]) + 1
                oy = min(ry1[a_i], ry1[b_i]) - max(ry0[a_i], ry0[b_i]) + 1
                if ox <= 0 or oy <= 0:
                    continue
                uww = int(max(rx1[a_i], rx1[b_i]) - min(rx0[a_i], rx0[b_i])) + 1
                uhh = int(max(ry1[a_i], ry1[b_i]) - min(ry0[a_i], ry0[b_i])) + 1
                if uww > MAXW or uhh > MAXW:
                    continue
                od, cd = (uww, uhh) if xmajor[a_i] else (uhh, uww)
                sepst = (nstrips(int(winH[a_i]), int(winW[a_i]))
                         + nstrips(int(winH[b_i]), int(winW[b_i]))
                         if not xmajor[a_i] else
                         nstrips(int(winW[a_i]), int(winH[a_i]))
                         + nstrips(int(winW[b_i]), int(winH[b_i])))
                save = sepst - nstrips(od, cd)
                if save > 0 and (bestp is None or save > bestp[0]):
                    bestp = (save, b_i)
            if bestp is not None:
                b_i = bestp[1]
                used.add(b_i)
                rect = (int(min(ry0[a_i], ry0[b_i])),
                        int(max(ry1[a_i], ry1[b_i])),
                        int(min(rx0[a_i], rx0[b_i])),
                        int(max(rx1[a_i], rx1[b_i])))
                # band constraint still holds? row-major unions: winH<=54
                ok = True
                if g < 6:
                    ok = (rect[1] <= ROW_BANDS[g % 3][0]
                          + ROW_BANDS[g % 3][1] - 1) and \
                         (rect[1] - rect[0] + 1 <= MAXW)
                if ok:
                    jobs.append((g, a_i, b_i, rect))
                    continue
                used.discard(b_i)
            jobs.append((g, a_i, -1,
                         (int(ry0[a_i]), int(ry1[a_i]),
                          int(rx0[a_i]), int(rx1[a_i]))))

    # ---- strip lists + a-factors per job; split into pieces ----
    # piece: (g, ra, rb, loc[int16 n], af[n, S, 28fp16])
    pieces = []
    for (g, ra, rb, rect) in jobs:
        qy0, qy1, qx0, qx1 = rect
        xm = bool(xmajor[ra])
        wH = qy1 - qy0 + 1
        wW = qx1 - qx0 + 1
        od, cd = (wW, wH) if xm else (wH, wW)
        ncd = -(-cd // S)
        n_r = od * ncd
        ods = np.repeat(np.arange(od), ncd)
        cds = np.tile(np.arange(ncd) * S, od)
        lv_, b_ = int(lvl[ra]), int(b[ra])
        if g < 6:
            rl, oy0 = 304, ROW_BANDS[g % 3][0]
            segoff = 0
        elif g < 12:
            rl, oy0 = 200, XCOL_BANDS[g % 3][0]
            segoff = 0
        elif g == 12:
            rl, oy0 = 152, 0
            segoff = b_ * SEG_SZ[1]
        else:
            rl, oy0 = (76, 0) if lv_ == 2 else (38, 0)
            segoff = (2 * SEG_SZ[2] + b_ * SEG_SZ[3]) if lv_ == 3 \
                else b_ * SEG_SZ[2]
        if xm:
            loc = segoff + (ods + qx0 - oy0) * rl + (cds + qy0)
        else:
            loc = segoff + (ods + qy0 - oy0) * rl + (cds + qx0)
        assert loc.min() >= 0 and loc.max() < GROUPS[g][1]
        af = np.zeros((n_r, S, 28), np.float16)
        ii = np.arange(S)
        if xm:
            yy = cds[:, None] + ii[None, :] + qy0          # [n, S]
            xx = np.broadcast_to((ods + qx0)[:, None], (n_r, S))
        else:
            yy = np.broadcast_to((ods + qy0)[:, None], (n_r, S))
            xx = cds[:, None] + ii[None, :] + qx0
        for r, side in [(ra, 0)] + ([(rb, 1)] if rb >= 0 else []):
            ryr = (yy - int(ry0[r])).reshape(-1)
            rxr = (xx - int(rx0[r])).reshape(-1)
            oky = (ryr >= 0) & (ryr < AROWS)
            okx = (rxr >= 0) & (rxr < AROWS)
            ayv = np.zeros((n_r * S, OUT), np.float16)
            axv = np.zeros((n_r * S, OUT), np.float16)
            ayv[oky] = Ay16[r][ryr[oky]]
            axv[okx] = Ax16[r][rxr[okx]]
            af[:, :, 7 * side:7 * side + 7] = ayv.reshape(n_r, S, OUT)
            af[:, :, 14 + 7 * side:21 + 7 * side] = axv.reshape(n_r, S, OUT)
        loc16 = loc.astype(np.int16)
        for p0 in range(0, n_r, 128):
            p1 = min(p0 + 128, n_r)
            pieces.append((g, ra, rb, loc16[p0:p1], af[p0:p1]))

    # ---- deal pieces to cores (LPT per group by strip count) ----
    per_core = [[[] for _ in range(NG)] for _ in range(N_CORES)]
    load_g = np.zeros((N_CORES, NG), np.int64)
    load = np.zeros(N_CORES, np.int64)
    ordp = sorted(range(len(pieces)), key=lambda p: -pieces[p][3].size)
    for p in ordp:
        g = pieces[p][0]
        c = int(np.argmin(load_g[:, g] * 1000 + load))
        per_core[c][g].append(p)
        w = pieces[p][3].size + 16           # +16 ~ packing overhead
        load_g[c, g] += w
        load[c] += w

    # ---- pack pieces into slots per core/group ----
    # slot: [(piece, side), ...] ; union pieces take both sides
    core_slots = [[[] for _ in range(NG)] for _ in range(N_CORES)]
    for c in range(N_CORES):
        for g in range(NG):
            uni = [p for p in per_core[c][g] if pieces[p][2] >= 0]
            sing = sorted([p for p in per_core[c][g] if pieces[p][2] < 0],
                          key=lambda p: -pieces[p][3].size)
            slots = [[(p, None)] for p in uni]
            i, j = 0, len(sing) - 1
            while i <= j:
                a_p = sing[i]
                if i == j:
                    slots.append([(a_p, 0)])
                    break
                b_p = sing[j]
                if pieces[a_p][3].size + pieces[b_p][3].size <= 128:
                    slots.append([(a_p, 0), (b_p, 1)])
                    i += 1
                    j -= 1
                else:
                    slots.append([(a_p, 0)])
                    i += 1
            core_slots[c][g] = slots

    schedule = tuple((g, max(len(core_slots[c][g]) for c in range(N_CORES)))
                     for g in range(NG)
                     if max(len(core_slots[c][g]) for c in range(N_CORES)))

    # ---- emit per-core streams ----
    idx_all, a_all, omap = [], [], []
    for c in range(N_CORES):
        idx_blocks, a_blocks, slotmap = [], [], []
        for g, nslots in schedule:
            slots = core_slots[c][g]
            for si in range(nslots):
                ib = np.zeros(128, np.int16)
                ab = np.zeros((128, S, 28), np.float16)
                ra_o = rb_o = -1
                if si < len(slots):
                    sp = 0
                    for (p, side) in slots[si]:
                        _, ra, rb, loc16, af = pieces[p]
                        n = loc16.size
                        ib[sp:sp + n] = loc16
                        if side is None or side == 0:
                            ab[sp:sp + n] = af
                        else:       # shift single job A-cols to B side
                            ab[sp:sp + n, :, 7:14] = af[:, :, 0:7]
                            ab[sp:sp + n, :, 21:28] = af[:, :, 14:21]
                        if side is None:
                            ra_o, rb_o = ra, rb
                        elif side == 0:
                            ra_o = ra
                        else:
                            rb_o = ra
                        sp += n
                    assert sp <= 128
                slotmap.append((ra_o, rb_o))
                idx_blocks.append(ib)
                a_blocks.append(ab.reshape(128, S * 28))
        stream = np.concatenate(idx_blocks)
        idx_all.append(np.tile(stream.reshape(-1, 16).T, (8, 1)))
        a_all.append(np.ascontiguousarray(np.concatenate(a_blocks, axis=1)))
        omap.append(slotmap)
    return table_flat, idx_all, a_all, omap, schedule


LAST_RESULT = None


def kernel(f0, f1, f2, f3, boxes, box_batch_idx):
    global LAST_RESULT
    table_flat, idx_all, a_all, omap, schedule = _host_prep(
        f0, f1, f2, f3, boxes, box_batch_idx)
    if schedule not in _nc_cache:
        _nc_cache[schedule] = _build_nc(schedule)
    nc = _nc_cache[schedule]
    in_maps = [{"table": table_flat, "idxs": idx_all[i], "afac": a_all[i]}
               for i in range(N_CORES)]
    res = bass_utils.run_bass_kernel_spmd(nc, in_maps,
                                          core_ids=list(range(N_CORES)))
    LAST_RESULT = res

    outfull = np.zeros((1024, 49, C), np.float32)
    for core in range(N_CORES):
        nslots = len(omap[core])
        r = np.asarray(res.results[core]["out"]).astype(np.float32)
        r = r.reshape(98, nslots, C)
        for slot, (ra, rb) in enumerate(omap[core]):
            if ra >= 0:
                outfull[ra] += r[0:49, slot]
            if rb >= 0:
                outfull[rb] += r[49:98, slot]
    return np.ascontiguousarray(
        outfull.transpose(0, 2, 1).reshape(1024, C, OUT, OUT))


# revision 23
# speedup vs baseline: 1.0893x; 1.0893x over previous
"""FPN ROIAlign pooler (nn_Pooler) on 8 trn2 cores — matmul formulation.

Strategy: data-parallel over RoIs. Host builds a channels-last fp16 pixel
table [px, 256ch] (row-major feature pyramid + an x-major copy of lvl0 for
tall windows). Per RoI the bounding rect of its bilinear taps becomes a
list of 6-px strips (one DMA descriptor each; dma_gather costs ~8.4ns per
descriptor on gpsimd, so 6-px strips balance descriptor count against
fetch padding). Overlapping same-level windows merge into union-rect jobs
(two RoIs sharing strips). Jobs split into pieces of <=128 strips; pieces
pack two-per-slot (one per 49-bin side). Every slot is one gathered chunk
[128 strips, 6*256] = 6 matmul sub-chunks [128 px, 256] with px on
partitions. ROIAlign weights are separable (W[px,bin] = Ay[y,by]*Ax[x,bx]);
host streams per-strip-sub factors [AyA|AyB|AxA|AxB] (4*7 fp16) and the
DVE forms the slot's 98-wide W on device. The PE array accumulates
pooled[98, 256] = W^T @ F in PSUM over the slot's 6 sub-chunks; the scalar
engine copies PSUM -> SBUF as fp16 and rows DMA out. Host adds piece
partial sums and reassembles [1024,256,7,7] in fp32.

int16 gather indices limit a gather group to <32768 px, so lvl0 (60800 px
per batch) is covered by 3 overlapping y-bands (windows with winH<=54) and
3 overlapping x-bands of the transposed copy (winH>54 implies winW<=17,
since window feature area <= 784 + margins). HW limits one dma_gather to
<=1024 descriptors = exactly one 8-slot wave.
"""
import numpy as np
from contextlib import ExitStack

from concourse import bacc, bass, mybir, tile, bass_utils

C = 256
N_CORES = 8
OUT = 7
S = 8               # strip length (px per gather descriptor)
LVL_HW = [(200, 304), (100, 152), (50, 76), (25, 38)]
SCALES = (0.25, 0.125, 0.0625, 0.03125)
SEG_SZ = [h * w for h, w in LVL_HW]           # px per (lvl, batch) segment
# segment order: (0,0),(0,1),(1,0),(1,1),(2,0),(2,1),(3,0),(3,1)
SEG_BASE = np.zeros((4, 2), np.int64)
_off = 0
for _l in range(4):
    for _b in range(2):
        SEG_BASE[_l, _b] = _off
        _off += SEG_SZ[_l]
TOTAL_PX = int(_off)                           # 161500
XSEG_BASE = [TOTAL_PX, TOTAL_PX + SEG_SZ[0]]   # x-major lvl0 copies
END_PAD = 32
TABLE_PX = TOTAL_PX + 2 * SEG_SZ[0] + END_PAD

ROW_BANDS = [(0, 107), (53, 107), (106, 94)]   # (row0, nrows), lvl0 W=304
XCOL_BANDS = [(0, 107), (90, 107), (180, 124)]  # (col0, ncols), lvl0 H=200
HMAX_ROW = 54        # row-major path requires winH <= 54
WMAX_X = 17          # x-major path requires winW <= 17

# gather groups: (table_base_px, n_px)
GROUPS = []
for _b in range(2):
    for _r0, _nr in ROW_BANDS:
        GROUPS.append((int(SEG_BASE[0, _b]) + _r0 * 304, _nr * 304))
for _b in range(2):
    for _c0, _ncol in XCOL_BANDS:
        GROUPS.append((XSEG_BASE[_b] + _c0 * 200, _ncol * 200))
GROUPS.append((int(SEG_BASE[1, 0]), 2 * SEG_SZ[1]))             # lvl1 both b
GROUPS.append((int(SEG_BASE[2, 0]), 2 * SEG_SZ[2] + 2 * SEG_SZ[3]))  # lvl2+3
NG = len(GROUPS)                               # 14
for _base, _npx in GROUPS:
    assert _npx <= 32768

MAXW = 124          # max window extent per side
AROWS = MAXW + S + 2

_nc_cache = {}


def _build_nc(schedule):
    """schedule: tuple of (g, n_slots)."""
    tot_slots = sum(n for _, n in schedule)
    tot_idxc = tot_slots * 8
    tot_acols = tot_slots * S * 28

    nc = bacc.Bacc("TRN2", target_bir_lowering=False, debug=False,
                   num_devices=N_CORES, num_swdge_queues=2)
    table_d = nc.dram_tensor("table", [TABLE_PX * C], mybir.dt.float16,
                             kind="ExternalInput")
    idx_d = nc.dram_tensor("idxs", [128, tot_idxc], mybir.dt.int16,
                           kind="ExternalInput")
    a_d = nc.dram_tensor("afac", [128, tot_acols], mybir.dt.float16,
                         kind="ExternalInput")
    # bin-major output: [98 bins, tot_slots * C] -> one 4KB-descriptor
    # DMA per wave instead of 98 tiny rows per slot
    out_d = nc.dram_tensor("out", [98, tot_slots * C], mybir.dt.float16,
                           kind="ExternalOutput")

    with tile.TileContext(nc) as tc, ExitStack() as ctx:
        sbi = ctx.enter_context(tc.tile_pool(name="sbi", bufs=6))
        sba = ctx.enter_context(tc.tile_pool(name="sba", bufs=6))
        sbd = ctx.enter_context(tc.tile_pool(name="sbd", bufs=6))
        sbw = ctx.enter_context(tc.tile_pool(name="sbw", bufs=6))
        sbo = ctx.enter_context(tc.tile_pool(name="sbo", bufs=4))
        psp = ctx.enter_context(tc.tile_pool(name="psp", bufs=8,
                                             space="PSUM"))

        # wave list in slot order, then interleave small (low PE work)
        # waves among full ones so the PE never starves
        waves = []
        ri = 0
        for g, nslots in schedule:
            for pos in range(0, nslots, 4):
                wch = min(4, nslots - pos)
                waves.append((g, ri, wch))
                ri += wch
        bigs = [w for w in waves if w[2] == 4]
        smalls = [w for w in waves if w[2] < 4]
        order = []
        bi = si = 0
        while bi < len(bigs) or si < len(smalls):
            for _ in range(2):
                if bi < len(bigs):
                    order.append(bigs[bi]); bi += 1
            if si < len(smalls):
                order.append(smalls[si]); si += 1

        for wv, (g, ri, wch) in enumerate(order):
            base_px, n_px = GROUPS[g]
            in_ap = bass.AP(tensor=table_d, offset=base_px * C,
                            ap=[[C, n_px], [1, S * C]])
            if True:
                idx_t = sbi.tile([128, 4 * 8], mybir.dt.int16)
                nc.default_dma_engine.dma_start(
                    out=idx_t[:, :wch * 8],
                    in_=idx_d.ap()[:, ri * 8:(ri + wch) * 8])
                dst_t = sbd.tile([128, 4, S * C], mybir.dt.float16)
                nc.gpsimd.dma_gather(
                    dst_t[:, :wch, :], in_ap, idx_t[:, :wch * 8],
                    wch * 128, wch * 128, S * C, elem_step=C)
                a_t = sba.tile([128, 4 * S, 28], mybir.dt.float16)
                nc.default_dma_engine.dma_start(
                    out=a_t[:, :wch * S, :],
                    in_=a_d.ap()[:, ri * S * 28:(ri + wch) * S * 28])
                w_t = sbw.tile([128, 4 * S, 2, 49], mybir.dt.float16)
                for j in range(2):
                    nc.vector.tensor_tensor(
                        out=w_t[:, :wch * S, j, :].rearrange(
                            "p c (a b) -> p c a b", a=7, b=7),
                        in0=a_t[:, :wch * S, 7 * j:7 * j + 7]
                        .unsqueeze(3).broadcast_to([128, wch * S, 7, 7]),
                        in1=a_t[:, :wch * S, 14 + 7 * j:21 + 7 * j]
                        .unsqueeze(2).broadcast_to([128, wch * S, 7, 7]),
                        op=mybir.AluOpType.mult)
                ot = sbo.tile([98, 4, C], mybir.dt.float16)
                for t in range(wch):
                    ps_t = psp.tile([98, 512], mybir.dt.float32)
                    for i in range(S):
                        nc.tensor.matmul(
                            ps_t[:, :C],
                            w_t[:, t * S + i, :, :].rearrange(
                                "p a b -> p (a b)"),
                            dst_t[:, t, i * C:(i + 1) * C],
                            start=(i == 0), stop=(i == S - 1))
                    nc.scalar.copy(out=ot[:, t, :], in_=ps_t[:, :C])
                # issue from the scalar queue: its wait on `ot` is already
                # satisfied there, keeping the sync queue free to prefetch
                # the next wave's idx/afac loads
                nc.scalar.dma_start(
                    out=out_d.ap()[:, ri * C:(ri + wch) * C],
                    in_=ot[:, :wch, :].rearrange("p a b -> p (a b)"))
        assert sum(w[2] for w in order) == tot_slots
    nc.compile()
    return nc


def _host_prep(f0, f1, f2, f3, boxes, bidx):
    boxes32 = np.asarray(boxes, np.float32)
    b = np.asarray(bidx).astype(np.int64)
    N = boxes32.shape[0]

    # level routing in strict fp32 (matches jax reference arithmetic)
    x1, y1, x2, y2 = (boxes32[:, k] for k in range(4))
    area = (x2 - x1 + np.float32(1.0)) * (y2 - y1 + np.float32(1.0))
    s = np.sqrt(area)
    lv = np.floor(np.float32(4.0) + np.log2(s / np.float32(224.0)
                                            + np.float32(1e-6)))
    lvl = (np.clip(lv, 2.0, 5.0)).astype(np.int64) - 2

    # channels-last flat fp16 table (+ x-major lvl0 copies for tall windows)
    segs = []
    for f in (f0, f1, f2, f3):
        fa = np.asarray(f, np.float32)
        for bb in range(2):
            segs.append(np.transpose(fa[bb], (1, 2, 0)).reshape(-1, C))
    fa0 = np.asarray(f0, np.float32)
    for bb in range(2):
        segs.append(np.transpose(fa0[bb], (2, 1, 0)).reshape(-1, C))
    segs.append(np.zeros((END_PAD, C), np.float32))
    table_flat = np.concatenate(segs, 0).astype(np.float16).reshape(-1)
    assert table_flat.size == TABLE_PX * C

    # bilinear sample geometry (fp64 tap positions, like proven baseline)
    scs = np.array(SCALES)[lvl]
    Wl = np.array([hw[1] for hw in LVL_HW])[lvl]
    Hl = np.array([hw[0] for hw in LVL_HW])[lvl]
    x1s = boxes32[:, 0].astype(np.float64) * scs
    y1s = boxes32[:, 1].astype(np.float64) * scs
    x2s = boxes32[:, 2].astype(np.float64) * scs
    y2s = boxes32[:, 3].astype(np.float64) * scs
    bin_w = np.maximum(x2s - x1s, 1.0) / OUT
    bin_h = np.maximum(y2s - y1s, 1.0) / OUT
    grid = (np.arange(OUT)[:, None] + np.array([0.25, 0.75])[None, :]).reshape(-1)
    xs = x1s[:, None] + bin_w[:, None] * grid[None, :]     # [N,14]
    ys = y1s[:, None] + bin_h[:, None] * grid[None, :]
    vx = (xs >= -1.0) & (xs <= Wl[:, None])
    vy = (ys >= -1.0) & (ys <= Hl[:, None])
    xc = np.clip(xs, 0.0, (Wl - 1)[:, None])
    yc = np.clip(ys, 0.0, (Hl - 1)[:, None])
    x0c = np.minimum(np.floor(xc).astype(np.int64), (Wl - 2)[:, None])
    y0c = np.minimum(np.floor(yc).astype(np.int64), (Hl - 2)[:, None])
    lx = xc - x0c
    ly = yc - y0c

    # window rects (inclusive px bounds [rx0..rx1] x [ry0..ry1])
    rx0 = x0c.min(axis=1); rx1 = x0c.max(axis=1) + 1
    ry0 = y0c.min(axis=1); ry1 = y0c.max(axis=1) + 1
    winW = (rx1 - rx0 + 1).astype(np.int64)
    winH = (ry1 - ry0 + 1).astype(np.int64)
    assert winW.max() <= MAXW and winH.max() <= MAXW
    npx = winH * winW

    # separable per-axis weight matrices Ay/Ax [N, AROWS, 7]
    nn = np.broadcast_to(np.arange(N)[:, None], (N, 14))
    sbin = np.broadcast_to((np.arange(14) // 2)[None, :], (N, 14))
    Ay = np.zeros((N, AROWS, OUT))
    Ax = np.zeros((N, AROWS, OUT))
    for kyw, rel, base, vv, rr in ((1.0 - ly, y0c, ry0, vy, Ay),
                                   (1.0 - lx, x0c, rx0, vx, Ax)):
        w0 = kyw * vv * 0.5
        w1 = (1.0 - kyw) * vv * 0.5
        np.add.at(rr, (nn, rel - base[:, None], sbin), w0)
        np.add.at(rr, (nn, rel - base[:, None] + 1, sbin), w1)
    Ay16 = Ay.astype(np.float16)
    Ax16 = Ax.astype(np.float16)

    # gather group per roi
    group = np.empty(N, np.int64)
    xmajor = np.zeros(N, np.bool_)
    for i in range(N):
        lv_, b_ = int(lvl[i]), int(b[i])
        if lv_ == 0:
            if winH[i] <= HMAX_ROW:
                band = 0 if ry1[i] <= 106 else (1 if ry1[i] <= 159 else 2)
                assert ry0[i] >= ROW_BANDS[band][0]
                assert ry1[i] <= ROW_BANDS[band][0] + ROW_BANDS[band][1] - 1
                group[i] = b_ * 3 + band
            else:
                assert winW[i] <= WMAX_X
                band = 0 if rx1[i] <= 106 else (1 if rx1[i] <= 196 else 2)
                assert rx0[i] >= XCOL_BANDS[band][0]
                assert rx1[i] <= XCOL_BANDS[band][0] + XCOL_BANDS[band][1] - 1
                group[i] = 6 + b_ * 3 + band
                xmajor[i] = True
        elif lv_ == 1:
            group[i] = 12
        else:
            group[i] = 13

    def nstrips(od, cd):
        return od * (-(-cd // S))

    # ---- union matching (global, per group; same (lvl,b) overlap) ----
    jobs = []   # (g, ra, rb, rect) ; rb=-1 single. rect=(qy0,qy1,qx0,qx1)
    for g in range(NG):
        ids = sorted([i for i in range(N) if group[i] == g],
                     key=lambda i: -npx[i])
        used = set()
        for a_i in ids:
            if a_i in used:
                continue
            used.add(a_i)
            bestp = None
            for b_i in ids:
                if b_i in used or lvl[a_i] != lvl[b_i] or b[a_i] != b[b_i]:
                    continue
                ox = min(rx1[a_i], rx1[b_i]) - max(rx0[a_i], rx0[b_i]) + 1
                oy = min(ry1[a_i], ry1[b_i]) - max(ry0[a_i], ry0[b_i]) + 1
                if ox <= 0 or oy <= 0:
                    continue
                uww = int(max(rx1[a_i], rx1[b_i]) - min(rx0[a_i], rx0[b_i])) + 1
                uhh = int(max(ry1[a_i], ry1[b_i]) - min(ry0[a_i], ry0[b_i])) + 1
                if uww > MAXW or uhh > MAXW:
                    continue
                od, cd = (uww, uhh) if xmajor[a_i] else (uhh, uww)
                sepst = (nstrips(int(winH[a_i]), int(winW[a_i]))
                         + nstrips(int(winH[b_i]), int(winW[b_i]))
                         if not xmajor[a_i] else
                         nstrips(int(winW[a_i]), int(winH[a_i]))
                         + nstrips(int(winW[b_i]), int(winH[b_i])))
                save = sepst - nstrips(od, cd)
                if save > 0 and (bestp is None or save > bestp[0]):
                    bestp = (save, b_i)
            if bestp is not None:
                b_i = bestp[1]
                used.add(b_i)
                rect = (int(min(ry0[a_i], ry0[b_i])),
                        int(max(ry1[a_i], ry1[b_i])),
                        int(min(rx0[a_i], rx0[b_i])),
                        int(max(rx1[a_i], rx1[b_i])))
                # band constraint still holds? row-major unions: winH<=54
                ok = True
                if g < 6:
                    ok = (rect[1] <= ROW_BANDS[g % 3][0]
                          + ROW_BANDS[g % 3][1] - 1) and \
                         (rect[1] - rect[0] + 1 <= MAXW)
                if ok:
                    jobs.append((g, a_i, b_i, rect))
                    continue
                used.discard(b_i)
            jobs.append((g, a_i, -1,
                         (int(ry0[a_i]), int(ry1[a_i]),
                          int(rx0[a_i]), int(rx1[a_i]))))

    # ---- strip lists + a-factors per job; split into pieces ----
    # piece: (g, ra, rb, loc[int16 n], af[n, S, 28fp16])
    pieces = []
    for (g, ra, rb, rect) in jobs:
        qy0, qy1, qx0, qx1 = rect
        xm = bool(xmajor[ra])
        wH = qy1 - qy0 + 1
        wW = qx1 - qx0 + 1
        od, cd = (wW, wH) if xm else (wH, wW)
        ncd = -(-cd // S)
        n_r = od * ncd
        ods = np.repeat(np.arange(od), ncd)
        cds = np.tile(np.arange(ncd) * S, od)
        lv_, b_ = int(lvl[ra]), int(b[ra])
        if g < 6:
            rl, oy0 = 304, ROW_BANDS[g % 3][0]
            segoff = 0
        elif g < 12:
            rl, oy0 = 200, XCOL_BANDS[g % 3][0]
            segoff = 0
        elif g == 12:
            rl, oy0 = 152, 0
            segoff = b_ * SEG_SZ[1]
        else:
            rl, oy0 = (76, 0) if lv_ == 2 else (38, 0)
            segoff = (2 * SEG_SZ[2] + b_ * SEG_SZ[3]) if lv_ == 3 \
                else b_ * SEG_SZ[2]
        if xm:
            loc = segoff + (ods + qx0 - oy0) * rl + (cds + qy0)
        else:
            loc = segoff + (ods + qy0 - oy0) * rl + (cds + qx0)
        assert loc.min() >= 0 and loc.max() < GROUPS[g][1]
        af = np.zeros((n_r, S, 28), np.float16)
        ii = np.arange(S)
        if xm:
            yy = cds[:, None] + ii[None, :] + qy0          # [n, S]
            xx = np.broadcast_to((ods + qx0)[:, None], (n_r, S))
        else:
            yy = np.broadcast_to((ods + qy0)[:, None], (n_r, S))
            xx = cds[:, None] + ii[None, :] + qx0
        for r, side in [(ra, 0)] + ([(rb, 1)] if rb >= 0 else []):
            ryr = (yy - int(ry0[r])).reshape(-1)
            rxr = (xx - int(rx0[r])).reshape(-1)
            oky = (ryr >= 0) & (ryr < AROWS)
            okx = (rxr >= 0) & (rxr < AROWS)
            ayv = np.zeros((n_r * S, OUT), np.float16)
            axv = np.zeros((n_r * S, OUT), np.float16)
            ayv[oky] = Ay16[r][ryr[oky]]
            axv[okx] = Ax16[r][rxr[okx]]
            af[:, :, 7 * side:7 * side + 7] = ayv.reshape(n_r, S, OUT)
            af[:, :, 14 + 7 * side:21 + 7 * side] = axv.reshape(n_r, S, OUT)
        loc16 = loc.astype(np.int16)
        for p0 in range(0, n_r, 128):
            p1 = min(p0 + 128, n_r)
            pieces.append((g, ra, rb, loc16[p0:p1], af[p0:p1]))

    # ---- deal pieces to cores (LPT per group by strip count) ----
    per_core = [[[] for _ in range(NG)] for _ in range(N_CORES)]
    load_g = np.zeros((N_CORES, NG), np.int64)
    load = np.zeros(N_CORES, np.int64)
    ordp = sorted(range(len(pieces)), key=lambda p: -pieces[p][3].size)
    for p in ordp:
        g = pieces[p][0]
        c = int(np.argmin(load_g[:, g] * 1000 + load))
        per_core[c][g].append(p)
        w = pieces[p][3].size + 16           # +16 ~ packing overhead
        load_g[c, g] += w
        load[c] += w

    # ---- pack pieces into slots per core/group ----
    # slot: [(piece, side), ...] ; union pieces take both sides
    core_slots = [[[] for _ in range(NG)] for _ in range(N_CORES)]
    for c in range(N_CORES):
        for g in range(NG):
            uni = [p for p in per_core[c][g] if pieces[p][2] >= 0]
            sing = sorted([p for p in per_core[c][g] if pieces[p][2] < 0],
                          key=lambda p: -pieces[p][3].size)
            slots = [[(p, None)] for p in uni]
            i, j = 0, len(sing) - 1
            while i <= j:
                a_p = sing[i]
                if i == j:
                    slots.append([(a_p, 0)])
                    break
                b_p = sing[j]
                if pieces[a_p][3].size + pieces[b_p][3].size <= 128:
                    slots.append([(a_p, 0), (b_p, 1)])
                    i += 1
                    j -= 1
                else:
                    slots.append([(a_p, 0)])
                    i += 1
            core_slots[c][g] = slots

    schedule = tuple((g, max(len(core_slots[c][g]) for c in range(N_CORES)))
                     for g in range(NG)
                     if max(len(core_slots[c][g]) for c in range(N_CORES)))

    # ---- emit per-core streams ----
    idx_all, a_all, omap = [], [], []
    for c in range(N_CORES):
        idx_blocks, a_blocks, slotmap = [], [], []
        for g, nslots in schedule:
            slots = core_slots[c][g]
            for si in range(nslots):
                ib = np.zeros(128, np.int16)
                ab = np.zeros((128, S, 28), np.float16)
                ra_o = rb_o = -1
                if si < len(slots):
                    sp = 0
                    for (p, side) in slots[si]:
                        _, ra, rb, loc16, af = pieces[p]
                        n = loc16.size
                        ib[sp:sp + n] = loc16
                        if side is None or side == 0:
                            ab[sp:sp + n] = af
                        else:       # shift single job A-cols to B side
                            ab[sp:sp + n, :, 7:14] = af[:, :, 0:7]
                            ab[sp:sp + n, :, 21:28] = af[:, :, 14:21]
                        if side is None:
                            ra_o, rb_o = ra, rb
                        elif side == 0:
                            ra_o = ra
                        else:
                            rb_o = ra
                        sp += n
                    assert sp <= 128
                slotmap.append((ra_o, rb_o))
                idx_blocks.append(ib)
                a_blocks.append(ab.reshape(128, S * 28))
        stream = np.concatenate(idx_blocks)
        idx_all.append(np.tile(stream.reshape(-1, 16).T, (8, 1)))
        a_all.append(np.ascontiguousarray(np.concatenate(a_blocks, axis=1)))
        omap.append(slotmap)
    return table_flat, idx_all, a_all, omap, schedule


LAST_RESULT = None


def kernel(f0, f1, f2, f3, boxes, box_batch_idx):
    global LAST_RESULT
    table_flat, idx_all, a_all, omap, schedule = _host_prep(
        f0, f1, f2, f3, boxes, box_batch_idx)
    if schedule not in _nc_cache:
        _nc_cache[schedule] = _build_nc(schedule)
    nc = _nc_cache[schedule]
    in_maps = [{"table": table_flat, "idxs": idx_all[i], "afac": a_all[i]}
               for i in range(N_CORES)]
    res = bass_utils.run_bass_kernel_spmd(nc, in_maps,
                                          core_ids=list(range(N_CORES)))
    LAST_RESULT = res

    outfull = np.zeros((1024, 49, C), np.float32)
    for core in range(N_CORES):
        nslots = len(omap[core])
        r = np.asarray(res.results[core]["out"]).astype(np.float32)
        r = r.reshape(98, nslots, C)
        for slot, (ra, rb) in enumerate(omap[core]):
            if ra >= 0:
                outfull[ra] += r[0:49, slot]
            if rb >= 0:
                outfull[rb] += r[49:98, slot]
    return np.ascontiguousarray(
        outfull.transpose(0, 2, 1).reshape(1024, C, OUT, OUT))
